# revision 8
# baseline (speedup 1.0000x reference)
import sys

if "/opt/trn_rl_repo" not in sys.path:
    sys.path.insert(0, "/opt/trn_rl_repo")

import numpy as np
from contextlib import ExitStack

import concourse.bass as bass
import concourse.tile as tile
from concourse import bacc, mybir
from concourse.bass_utils import run_bass_kernel_spmd

# Problem shapes (hardcoded per contract)
B = 8192      # batch
C = 4096      # channels
KTAP = 5      # banded window size
HALF = KTAP // 2
N_CORES = 8
B_SH = B // N_CORES          # 1024 batch rows per core
NCH = 512                    # matmul moving free dim (one PSUM bank of fp32)
NCHUNKS = B_SH // NCH        # 2

# Channel-block structure: out = A @ x where A is [C, C] banded (bw=5).
# Each block does one K=128 matmul producing M output channels:
#   block 0:    out [0, 126),     in [0, 128)
#   block i:    out [c0, c0+124), in [c0-2, c0+126),  c0 = 126 + 124*(i-1)
#   block 32:   out [3970, 4096), in [3968, 4096)
def _block_layout():
    outs, ms, ins = [], [], []
    outs.append(0); ms.append(126); ins.append(0)
    c0 = 126
    while c0 + 124 <= C - 126:
        outs.append(c0); ms.append(124); ins.append(c0 - 2)
        c0 += 124
    outs.append(c0); ms.append(C - c0); ins.append(C - 128)
    return outs, ms, ins

OUT_STARTS, M_LIST, IN_STARTS = _block_layout()
NBLK = len(OUT_STARTS)
assert sum(M_LIST) == C and NBLK == 33 and M_LIST[-1] == 126

_COMPILED = None  # (nc,) cached across calls


def _build_program():
    nc = bacc.Bacc(
        "TRN2",
        target_bir_lowering=False,
        debug=False,
        enable_asserts=False,
    )
    f32 = mybir.dt.float32
    xT_t = nc.dram_tensor("xT", [C, B_SH], f32, kind="ExternalInput").ap()
    A_t = nc.dram_tensor("A", [128, C], f32, kind="ExternalInput").ap()
    bc_t = nc.dram_tensor("bc", [128, NBLK], f32, kind="ExternalInput").ap()
    o_t = nc.dram_tensor("outT", [C, B_SH], f32, kind="ExternalOutput").ap()

    with tile.TileContext(nc) as tc:
        with ExitStack() as ctx:
            const_pool = ctx.enter_context(tc.tile_pool(name="const", bufs=1))
            x_pool = ctx.enter_context(tc.tile_pool(name="x", bufs=16))
            ps_pool = ctx.enter_context(tc.tile_pool(name="ps", bufs=8, space="PSUM"))
            o_pool = ctx.enter_context(tc.tile_pool(name="o", bufs=8))

            A_tile = const_pool.tile([128, C], f32)
            nc.sync.dma_start(A_tile[:], A_t[:])
            bc_tile = const_pool.tile([128, NBLK], f32)
            nc.sync.dma_start(bc_tile[:], bc_t[:])

            for blk in range(NBLK):
                os_, m, is_ = OUT_STARTS[blk], M_LIST[blk], IN_STARTS[blk]
                xt = x_pool.tile([128, B_SH], f32)
                nc.sync.dma_start(xt[:], xT_t[is_ : is_ + 128, :])
                ot = o_pool.tile([128, B_SH], f32)
                bias_ap = bc_tile[0:m, blk : blk + 1]
                for ch in range(NCHUNKS):
                    csl = bass.ds(ch * NCH, NCH)
                    ps = ps_pool.tile([128, NCH], f32)
                    nc.tensor.matmul(
                        ps[:m, :],
                        A_tile[:, os_ : os_ + m],
                        xt[:, csl],
                        start=True,
                        stop=True,
                    )
                    if (blk * NCHUNKS + ch) % 2 == 0:
                        nc.scalar.add(ot[:m, csl], ps[:m, :], bias_ap)
                    else:
                        nc.vector.tensor_scalar_add(ot[:m, csl], ps[:m, :], bias_ap)
                nc.gpsimd.dma_start(o_t[os_ : os_ + m, :], ot[:m, :])

    nc.compile()
    return nc


def _host_prep(x, W, b):
    # xT: [C, B] channel-major so per-core tiles are contiguous in DRAM
    xT = np.ascontiguousarray(x.T)
    # A_all[j, c] = W[c, cin - c + 2] with cin = IN_STARTS[blk(c)] + j
    A_all = np.zeros((128, C), dtype=np.float32)
    cs = np.arange(C)
    blk_of_c = np.zeros(C, dtype=np.int64)
    for blk in range(NBLK):
        blk_of_c[OUT_STARTS[blk] : OUT_STARTS[blk] + M_LIST[blk]] = blk
    in_start_of_c = np.array(IN_STARTS)[blk_of_c]
    for t in range(KTAP):
        cin = cs + t - HALF                     # input channel for tap t
        valid = (cin >= 0) & (cin < C)
        j = cin - in_start_of_c                 # row in A_all
        valid &= (j >= 0) & (j < 128)
        A_all[j[valid], cs[valid]] = W[cs[valid], t]
    bcols = np.zeros((128, NBLK), dtype=np.float32)
    for blk in range(NBLK):
        os_, m = OUT_STARTS[blk], M_LIST[blk]
        bcols[:m, blk] = b[os_ : os_ + m]
    return xT, A_all, bcols


def _run(x, W, b, trace=False, trace_kwargs=None):
    global _COMPILED
    if _COMPILED is None:
        _COMPILED = _build_program()
    nc = _COMPILED

    x = np.asarray(x, dtype=np.float32)
    W = np.asarray(W, dtype=np.float32)
    b = np.asarray(b, dtype=np.float32)
    xT, A_all, bcols = _host_prep(x, W, b)

    in_maps = []
    for i in range(N_CORES):
        shard = np.ascontiguousarray(xT[:, i * B_SH : (i + 1) * B_SH])
        in_maps.append({"xT": shard, "A": A_all, "bc": bcols})

    res = run_bass_kernel_spmd(
        nc,
        in_maps,
        core_ids=list(range(N_CORES)),
        trace=trace,
        **(trace_kwargs or {}),
    )
    outT = np.empty((C, B), dtype=np.float32)
    for i in range(N_CORES):
        outT[:, i * B_SH : (i + 1) * B_SH] = res.results[i]["outT"]
    out = np.ascontiguousarray(outT.T)
    return out, res


def kernel(x, W, b):
    out, _ = _run(x, W, b, trace=False)
    return out
